# revision 9
# baseline (speedup 1.0000x reference)
"""ChebNetConv (K=4) Bass kernel for 8 trn2 NeuronCores.

Strategy (1D row partitioning per sharding hint):
  - Nodes sharded across 8 cores (12500 rows each). Each SpMM step computes
    the core's own output rows; full neighbor tables (x / T1 / T2) are
    available to every core (x as replicated input; T1/T2 via AllGather).
  - Neighbor tables are split into four QUARTERS (by producing dest-block
    ranges, block-aligned) with separate AllGathers fired as soon as each
    quarter's rows are evicted (after superblocks 2/5/8/12).  Chunk c of
    the next step's gathers reads quarter-table c, so only the last
    quarter can stall, and superblock PAIRS process chunks {0,1,2} of
    both supers before chunk {3} to cover that latency.
  - SpMM core: edges grouped by (dest block of 128 rows, src quarter) and
    padded to batches of 128.  Per (superblock, chunk) region: dma_gather
    pulls source rows (256B bf16) into SBUF G[128e, nb, 128f]; selector
    tiles S[128e, 128d, nb] (Laplacian values at (e, dest-in-block)) are
    built ON-CHIP by two batched DVE passes (2x perf mode: q innermost,
    materialized iota constant):
        S = is_equal(iotaC, dst_bcast) * val_bcast
    PE matmuls accumulate S[:,:,q].T @ G[:,q,:] into dest-block PSUM.
  - Chebyshev recurrence (T2 = 2*L@T1 - T0) fused into batched PSUM
    eviction (one scalar_tensor_tensor per 4-block PSUM bank); bf16
    copies feed the AllGathers; T2/T3 skip the f32 DRAM round trip.
  - Final linear runs inline with step 3 (per super-pair): cheb tiles are
    loaded via HWDGE DMA-transpose from the bf16 shards (no PE transpose),
    K matmuls in bf16 against W slices accumulate in PSUM, bias added on
    DVE via partition-broadcast.
"""

import itertools

import numpy as np

import concourse.bacc as bacc
import concourse.bass as bass
import concourse.mybir as mybir
import concourse.tile as tile
from concourse import bass_utils
from concourse.bass import ds
from concourse.masks import make_identity

P = 128


class Cfg:
    def __init__(self, n_nodes=100000, f=128, k=4, cores=8, superb=8):
        assert n_nodes % cores == 0
        self.N = n_nodes
        self.F = f
        self.K = k
        self.CORES = cores
        self.RPC = n_nodes // cores            # rows per core
        self.NBLK = -(-self.RPC // P)          # dest blocks per core
        self.SUPER = superb                    # dest blocks per super-block
        self.NSUPER = -(-self.NBLK // superb)
        # quarter split (block-aligned) for pipelined AllGathers
        self.QBLK0 = [0, 24, 48, 72]           # first block of each quarter
        self.QBLK1 = [24, 48, 72, self.NBLK]   # one-past-last block
        self.QROWS = [min(b1 * P, self.RPC) - b0 * P
                      for b0, b1 in zip(self.QBLK0, self.QBLK1)]  # local rows
        self.QSTART = [b0 * P for b0 in self.QBLK0]
        self.TQ = [cores * r for r in self.QROWS]  # quarter-table rows
        for t in self.TQ:
            assert t <= 32767                  # int16 gather-idx limit
        self.NCHUNK = 4                        # chunk c == quarter c
        # AG for quarter q fires once superblock AG_SUPER[q] has evicted
        self.AG_SUPER = [2, 5, 8, self.NSUPER - 1]
        # superblock pairs (chunk phases {0,1,2} then {3} within a pair)
        self.PAIRS = [tuple(range(i, min(i + 2, self.NSUPER)))
                      for i in range(0, self.NSUPER, 2)]

    def blocks_of(self, s):
        return range(s * self.SUPER, min(self.NBLK, (s + 1) * self.SUPER))

    def nrows_of(self, b):
        return min(P, self.RPC - b * P)

    def quarter_of(self, b):
        for q in range(4):
            if b < self.QBLK1[q]:
                return q
        raise AssertionError(b)

    def map_cols(self, cols):
        """Map original node ids -> (chunk, idx-within-chunk) in the
        quarter-table layout."""
        o = cols // self.RPC
        loc = cols % self.RPC
        chk = np.zeros(len(cols), dtype=np.int64)
        src = np.zeros(len(cols), dtype=np.int64)
        for q in range(4):
            m = (loc >= self.QSTART[q]) & (loc < self.QSTART[q] + self.QROWS[q])
            chk[m] = q
            src[m] = o[m] * self.QROWS[q] + (loc[m] - self.QSTART[q])
        return chk, src

    def perm_quarters(self, full):
        """Split a [N, F] array into the 4 permuted quarter-table layouts."""
        outs = []
        for q in range(4):
            outs.append(np.ascontiguousarray(np.concatenate(
                [full[o * self.RPC + self.QSTART[q]:
                      o * self.RPC + self.QSTART[q] + self.QROWS[q]]
                 for o in range(self.CORES)], axis=0)))
        return outs


def preprocess(cfg, rows, cols, vals):
    """Build per-core gather-index and (dst, val) selector streams.

    Returns (meta, per_core) where meta has compile-time batch counts
    (identical across cores) and per_core[c] = dict of input arrays.
    """
    rows = np.asarray(rows).astype(np.int64)
    cols = np.asarray(cols).astype(np.int64)
    vals = np.asarray(vals).astype(np.float32)

    core = rows // cfg.RPC
    loc = rows % cfg.RPC
    blk = loc // P
    dst = loc % P
    chk, src = cfg.map_cols(cols)

    counts = np.zeros((cfg.CORES, cfg.NBLK, cfg.NCHUNK), dtype=np.int64)
    np.add.at(counts, (core, blk, chk), 1)
    NB = np.maximum(1, -(-counts.max(axis=0) // P))  # [NBLK, NCHUNK] batches
    # force even per-(super, chunk) batch counts (4B-aligned DVE runs)
    for s in range(cfg.NSUPER):
        bl = list(cfg.blocks_of(s))
        for c in range(cfg.NCHUNK):
            if sum(NB[b, c] for b in bl) % 2:
                NB[bl[-1], c] += 1

    # slot (b, c) capacity NB[b,c]*128; slot start offsets in padded edge space,
    # ordered (super, chunk, block-in-super, batch)
    slot_start = np.zeros((cfg.NBLK, cfg.NCHUNK), dtype=np.int64)
    call_start = {}          # (s, c) -> padded-edge offset of the gather call
    call_nbatch = {}         # (s, c) -> total batches in call
    off = 0
    for s in range(cfg.NSUPER):
        for c in range(cfg.NCHUNK):
            call_start[(s, c)] = off
            nb = 0
            for b in cfg.blocks_of(s):
                slot_start[b, c] = off
                off += NB[b, c] * P
                nb += NB[b, c]
            call_nbatch[(s, c)] = nb
    tot_pad = off

    meta = dict(NB=NB, call_start=call_start, call_nbatch=call_nbatch,
                tot_pad=tot_pad)

    import ml_dtypes
    per_core = []
    for cid in range(cfg.CORES):
        m = core == cid
        key = (blk[m] * cfg.NCHUNK + chk[m])
        order = np.argsort(key, kind="stable")
        kb, kc, ksrc, kdst, kval = (blk[m][order], chk[m][order],
                                    src[m][order], dst[m][order],
                                    vals[m][order])
        # rank within slot
        cnt = counts[cid].reshape(-1)
        slot_flat = kb * cfg.NCHUNK + kc
        starts = np.zeros(cfg.NBLK * cfg.NCHUNK, dtype=np.int64)
        starts[1:] = np.cumsum(cnt)[:-1]
        rank = np.arange(len(kb)) - starts[slot_flat]
        pos = slot_start.reshape(-1)[slot_flat] + rank  # padded global position

        idx_flat = np.zeros(tot_pad, dtype=np.int16)
        idx_flat[pos] = ksrc.astype(np.int16)
        # compact selector stream: per padded edge (dst, val) bf16 pairs
        dv = np.zeros((tot_pad, 2), dtype=ml_dtypes.bfloat16)
        dv[pos, 0] = kdst.astype(ml_dtypes.bfloat16)
        dv[pos, 1] = kval.astype(ml_dtypes.bfloat16)

        # idx DMA layout: per call, [128, 8*nb] with idx j at
        # [16g + j%16, j//16] for replica groups g=0..7
        idx_parts = []
        dv_parts = []
        for s in range(cfg.NSUPER):
            for c in range(cfg.NCHUNK):
                o = call_start[(s, c)]
                nb = call_nbatch[(s, c)]
                iv = idx_flat[o:o + nb * P]            # [nb*128]
                arr = iv.reshape(-1, 16).T             # [16, 8*nb]
                idx_parts.append(np.tile(arr, (8, 1)).reshape(-1))
                # dv layout per call: [128p, 2, nb] (q innermost)
                dvv = dv[o:o + nb * P].reshape(nb, P, 2)
                dv_parts.append(np.ascontiguousarray(
                    dvv.transpose(1, 2, 0)).reshape(-1))
        per_core.append(dict(
            idx_all=np.concatenate(idx_parts),
            dv_all=np.concatenate(dv_parts),
        ))
    return meta, per_core


def emulate(cfg, meta, per_core, full_tab):
    """Numpy emulation of the on-device SpMM. full_tab: [N, F] table in
    ORIGINAL node order. Returns per-core [RPC, F] segment sums."""
    NB = meta["NB"]
    chunk_tabs = cfg.perm_quarters(full_tab)
    outs = []
    for cid in range(cfg.CORES):
        pc = per_core[cid]
        out = np.zeros((cfg.RPC, cfg.F), dtype=np.float32)
        iofs = 0
        sofs = 0
        for s in range(cfg.NSUPER):
            for c in range(cfg.NCHUNK):
                nb = meta["call_nbatch"][(s, c)]
                w8 = nb * 8
                idx_tile = pc["idx_all"][iofs:iofs + 128 * w8].reshape(128, w8)
                iofs += 128 * w8
                n = nb * P
                unwrapped = idx_tile[:16, :].T.reshape(-1)[:n].astype(np.int64)
                g = chunk_tabs[c][unwrapped]             # [n, F]
                g = g.reshape(nb, P, cfg.F)
                dvt = pc["dv_all"][sofs:sofs + 128 * 2 * nb].reshape(128, 2, nb)
                sofs += 128 * 2 * nb
                q0 = 0
                for b in cfg.blocks_of(s):
                    for q in range(NB[b, c]):
                        dd = dvt[:, 0, q0 + q].astype(np.int64)
                        vv = dvt[:, 1, q0 + q].astype(np.float32)
                        S = np.zeros((P, P), dtype=np.float32)
                        S[np.arange(P), dd] = vv
                        G = g[q0 + q].astype(np.float32)
                        out[b * P:b * P + cfg.nrows_of(b), :] += \
                            (S.T @ G)[:cfg.nrows_of(b)]
                    q0 += NB[b, c]
        outs.append(out)
    return outs


def build(cfg, meta):
    """Build the Bass program. Returns nc."""
    f32 = mybir.dt.float32
    bf16 = mybir.dt.bfloat16
    nc = bacc.Bacc("TRN2", target_bir_lowering=False, debug=False,
                   num_devices=cfg.CORES, num_swdge_queues=4)

    x_tabq = [nc.dram_tensor(f"x_tab{q}", [cfg.TQ[q], cfg.F], bf16,
                             kind="ExternalInput") for q in range(4)]
    x_shard = nc.dram_tensor("x_shard", [cfg.RPC, cfg.F], f32,
                             kind="ExternalInput")
    x_bsh = nc.dram_tensor("x_bsh", [cfg.RPC, cfg.F], bf16,
                           kind="ExternalInput")
    idx_in = nc.dram_tensor("idx_all", [meta["tot_pad"] * 8], mybir.dt.int16,
                            kind="ExternalInput")
    dv_in = nc.dram_tensor("dv_all", [meta["tot_pad"] * 2], bf16,
                           kind="ExternalInput")
    w_in = nc.dram_tensor("w_lhsT", [cfg.F, cfg.K * cfg.F], bf16,
                          kind="ExternalInput")
    b_in = nc.dram_tensor("b_row", [1, cfg.F], f32, kind="ExternalInput")
    out_shard = nc.dram_tensor("out_shard", [cfg.RPC, cfg.F], f32,
                               kind="ExternalOutput")

    rg = [list(range(cfg.CORES))]
    qrr = itertools.count()  # global gather-queue round robin
    NBMAX = max(meta["call_nbatch"].values())

    with tile.TileContext(nc) as tc:
        with tc.tile_pool(name="dram", bufs=1, space="DRAM") as dram:
            t1_shard = dram.tile([cfg.RPC, cfg.F], f32, tag="t1s")
            t1_bq = [dram.tile([cfg.QROWS[q], cfg.F], bf16, tag=f"t1b{q}",
                               name=f"t1b{q}") for q in range(4)]
            t2_bq = [dram.tile([cfg.QROWS[q], cfg.F], bf16, tag=f"t2b{q}",
                               name=f"t2b{q}") for q in range(4)]
            t3_bsh = dram.tile([cfg.RPC, cfg.F], bf16, tag="t3b")
            t1_tabq = [dram.tile([cfg.TQ[q], cfg.F], bf16, tag=f"t1t{q}",
                                 name=f"t1t{q}", addr_space="Shared")
                       for q in range(4)]
            t2_tabq = [dram.tile([cfg.TQ[q], cfg.F], bf16, tag=f"t2t{q}",
                                 name=f"t2t{q}", addr_space="Shared")
                       for q in range(4)]

            def ag(bsh, tab):
                nc.gpsimd.collective_compute(
                    "AllGather", mybir.AluOpType.bypass, replica_groups=rg,
                    ins=[bsh[:].opt()], outs=[tab[:].opt()])

            with (
                tc.tile_pool(name="const", bufs=1) as constp,
                tc.tile_pool(name="gpool", bufs=4) as gpool,
                tc.tile_pool(name="spool", bufs=3) as spool,
                tc.tile_pool(name="ipool", bufs=5) as ipool,
                tc.tile_pool(name="dvpool", bufs=5) as dvpool,
                tc.tile_pool(name="psum", bufs=2, space="PSUM") as pspool,
                tc.tile_pool(name="ev", bufs=4) as evpool,
                tc.tile_pool(name="fconst", bufs=1) as fconst,
                tc.tile_pool(name="ftrans", bufs=8) as ftrans,
                tc.tile_pool(name="fpsum", bufs=2, space="PSUM") as fpsum,
                tc.tile_pool(name="fout", bufs=3) as foutp,
            ):
                iotaC = constp.tile([P, P, NBMAX], bf16)
                nc.gpsimd.iota(iotaC[:], pattern=[[1, P], [0, NBMAX]], base=0,
                               channel_multiplier=0,
                               allow_small_or_imprecise_dtypes=True)
                wt = fconst.tile([cfg.F, cfg.K, cfg.F], bf16)
                nc.sync.dma_start(wt[:], w_in[:].rearrange(
                    "f (k o) -> f k o", k=cfg.K))
                brow = fconst.tile([1, cfg.F], f32)
                nc.sync.dma_start(brow[:], b_in[:])
                ones = fconst.tile([1, P], f32)
                nc.vector.memset(ones[:], 1.0)

                fin = dict(ftrans=ftrans, fpsum=fpsum, foutp=foutp, wt=wt,
                           brow=brow, ones=ones, out_shard=out_shard)
                for step in (1, 2, 3):
                    tabs = {1: [t[:] for t in x_tabq],
                            2: [t[:] for t in t1_tabq],
                            3: [t[:] for t in t2_tabq]}[step]
                    prev = {1: None, 2: x_shard, 3: t1_shard}[step]
                    dst = {1: t1_shard, 2: None, 3: None}[step]
                    bq = {1: t1_bq, 2: t2_bq, 3: None}[step]
                    hooks = {}
                    if step == 1:
                        hooks = {q: (lambda q=q: ag(t1_bq[q], t1_tabq[q]))
                                 for q in range(4)}
                    elif step == 2:
                        hooks = {q: (lambda q=q: ag(t2_bq[q], t2_tabq[q]))
                                 for q in range(4)}
                    fin_step = fin if step == 3 else None
                    fin_srcs = ([x_bsh[:], t1_bq, t2_bq, t3_bsh[:]]
                                if step == 3 else None)
                    spmm_step(cfg, meta, nc, tc, gpool, spool, ipool, dvpool,
                              pspool, evpool, idx_in, dv_in, iotaC, tabs,
                              prev, dst, qrr, bq, t3_bsh if step == 3 else None,
                              hooks, fin_step, fin_srcs)

    nc.compile()
    return nc


def final_linear_block(cfg, nc, fin, fin_srcs, b):
    """Inline final linear for one dest block (step-3 tail)."""
    f32 = mybir.dt.float32
    bf16 = mybir.dt.bfloat16
    nrows = cfg.nrows_of(b)
    r0 = b * P
    opsum = fin["fpsum"].tile([P, cfg.F], f32, tag="opsum")
    for k in range(cfg.K):
        src = fin_srcs[k]
        if isinstance(src, list):  # quarter tiles
            q = cfg.quarter_of(b)
            reg = src[q][(b - cfg.QBLK0[q]) * P:
                         (b - cfg.QBLK0[q]) * P + nrows, :]
        else:
            reg = src[r0:r0 + nrows, :]
        cT = fin["ftrans"].tile([cfg.F, P], bf16, tag="cT")
        # scalar (ACT) HWDGE ring: keeps the transpose stream off the sync
        # ring that carries the idx/dv/pv loads (head-of-line blocking).
        if nrows == P:
            nc.scalar.dma_start_transpose(cT[:, :nrows], reg)
        else:
            nc.scalar.dma_start(cT[:, :nrows], reg.rearrange("a b -> b a"))
        nc.tensor.matmul(opsum[:nrows, :], cT[:, :nrows],
                         fin["wt"][:, k, :], start=(k == 0), stop=False)
    nc.tensor.matmul(opsum[:nrows, :], fin["ones"][:1, :nrows],
                     fin["brow"][:1, :], start=False, stop=True)
    ot = fin["foutp"].tile([P, cfg.F], f32, tag="ot")
    nc.vector.tensor_copy(ot[:nrows, :], opsum[:nrows, :])
    nc.scalar.dma_start(fin["out_shard"][r0:r0 + nrows, :], ot[:nrows, :])


def spmm_step(cfg, meta, nc, tc, gpool, spool, ipool, dvpool, pspool, evpool,
              idx_in, dv_in, iotaC, tabs, prev, dst, qrr, bq, t3b, hooks,
              fin, fin_srcs):
    NB = meta["NB"]
    f32 = mybir.dt.float32
    bf16 = mybir.dt.bfloat16
    eq = mybir.AluOpType.is_equal
    mul = mybir.AluOpType.mult
    sub = mybir.AluOpType.subtract
    base_i = {}
    base_s = {}
    off_i = 0
    off_s = 0
    for s in range(cfg.NSUPER):
        for c in range(cfg.NCHUNK):
            nb = meta["call_nbatch"][(s, c)]
            base_i[(s, c)] = off_i
            base_s[(s, c)] = off_s
            off_i += P * nb * 8
            off_s += P * nb * 2

    def do_chunk(s, c, ps):
        blocks = list(cfg.blocks_of(s))
        src = tabs[c]
        nb = meta["call_nbatch"][(s, c)]
        w8 = nb * 8
        iofs = base_i[(s, c)]
        sofs = base_s[(s, c)]
        ix = ipool.tile([P, w8], mybir.dt.int16, tag="ix")
        nc.sync.dma_start(
            ix[:], idx_in[iofs:iofs + P * w8].rearrange("(p w) -> p w", p=P))
        dv = dvpool.tile([P, 2, nb], bf16, tag="dv")
        nc.sync.dma_start(
            dv[:], dv_in[sofs:sofs + P * nb * 2].rearrange(
                "(p t b) -> p t b", p=P, t=2))
        g = gpool.tile([P, nb, cfg.F], bf16, tag="G")
        MAXB = 16
        for b0 in range(0, nb, MAXB):
            b1 = min(nb, b0 + MAXB)
            nc.gpsimd.dma_gather(
                g[:, b0:b1, :], src, ix[:, b0 * 8:b1 * 8],
                (b1 - b0) * P, (b1 - b0) * P, cfg.F,
                single_packet=(b1 - b0) <= 8, queue_num=next(qrr) % 4)
        # batched on-chip selector build (2 DVE passes, q innermost -> 2x)
        sl = spool.tile([P, P, nb], bf16, tag="S")
        nc.vector.tensor_tensor(
            sl[:], iotaC[:, :, :nb], dv[:, 0:1, :].broadcast_to([P, P, nb]),
            op=eq)
        nc.vector.tensor_tensor(
            sl[:], sl[:], dv[:, 1:2, :].broadcast_to([P, P, nb]), op=mul)
        q0 = 0
        for bi, b in enumerate(blocks):
            pt = ps[bi // 4][:, bi % 4, :]
            # one accumulation group per PSUM bank: start clears
            # has_written bank-wide, so only the first matmul into the
            # bank may set it; per-element has_written handles the
            # disjoint block slices.
            last_in_bank = bi % 4 == 3 or bi == len(blocks) - 1
            for q in range(NB[b, c]):
                nc.tensor.matmul(
                    pt, sl[:, :, q0 + q], g[:, q0 + q, :],
                    start=(c == 0 and q == 0 and bi % 4 == 0),
                    stop=(c == cfg.NCHUNK - 1 and q == NB[b, c] - 1
                          and last_in_bank),
                    skip_group_check=True)
            q0 += NB[b, c]

    def evict_super(s, ps):
        blocks = list(cfg.blocks_of(s))
        gi = 0
        while gi < len(blocks):
            grp = blocks[gi:gi + 4]
            pst = ps[gi // 4]
            full = len(grp) == 4 and all(cfg.nrows_of(b) == P for b in grp)
            b0 = grp[0]
            r0 = b0 * P
            if full:
                ev4 = evpool.tile([P, 4, cfg.F], f32, tag="ev4")
                if prev is None:
                    nc.vector.tensor_copy(ev4[:], pst[:])
                else:
                    pv4 = evpool.tile([P, 4, cfg.F], f32, tag="pv4")
                    nc.sync.dma_start(pv4[:], prev[r0:r0 + 4 * P, :].rearrange(
                        "(g p) f -> p g f", p=P))
                    nc.vector.scalar_tensor_tensor(
                        ev4[:], pst[:], 2.0, pv4[:], op0=mul, op1=sub)
                if dst is not None:
                    nc.scalar.dma_start(
                        dst[r0:r0 + 4 * P, :].rearrange(
                            "(g p) f -> p g f", p=P), ev4[:])
                evb = evpool.tile([P, 4, cfg.F], bf16, tag="evb4")
                nc.vector.tensor_copy(evb[:], ev4[:])
                if bq is not None:
                    qt = cfg.quarter_of(b0)
                    tr0 = r0 - cfg.QSTART[qt]
                    tgt = bq[qt]
                else:
                    qt = None
                    tr0 = r0
                    tgt = t3b
                nc.scalar.dma_start(
                    tgt[tr0:tr0 + 4 * P, :].rearrange(
                        "(g p) f -> p g f", p=P), evb[:])
                gi += 4
            else:
                for j, b in enumerate(grp):
                    pt = pst[:, j, :]
                    nrows = cfg.nrows_of(b)
                    r0 = b * P
                    ev = evpool.tile([P, cfg.F], f32, tag="ev")
                    if prev is None:
                        nc.vector.tensor_copy(ev[:nrows, :], pt[:nrows, :])
                    else:
                        pv = evpool.tile([P, cfg.F], f32, tag="pv")
                        nc.sync.dma_start(pv[:nrows, :],
                                          prev[r0:r0 + nrows, :])
                        nc.vector.scalar_tensor_tensor(
                            ev[:nrows, :], pt[:nrows, :], 2.0, pv[:nrows, :],
                            op0=mul, op1=sub)
                    if dst is not None:
                        nc.scalar.dma_start(dst[r0:r0 + nrows, :],
                                            ev[:nrows, :])
                    evb = evpool.tile([P, cfg.F], bf16, tag="evb")
                    nc.vector.tensor_copy(evb[:nrows, :], ev[:nrows, :])
                    if bq is not None:
                        qt = cfg.quarter_of(b)
                        tgt, tr0 = bq[qt], r0 - cfg.QSTART[qt]
                    else:
                        tgt, tr0 = t3b, r0
                    nc.scalar.dma_start(tgt[tr0:tr0 + nrows, :],
                                        evb[:nrows, :])
                gi += len(grp)

    fired = set()
    for pair in cfg.PAIRS:
        pair_ps = {}
        for s in pair:
            blocks = list(cfg.blocks_of(s))
            pair_ps[s] = [pspool.tile([P, 4, cfg.F], f32, tag=f"ps{i}",
                                      name=f"ps{i}_{s}")
                          for i in range(-(-len(blocks) // 4))]
        for c in (0, 1, 2):
            for s in pair:
                do_chunk(s, c, pair_ps[s])
        for s in pair:
            do_chunk(s, 3, pair_ps[s])
        for s in pair:
            evict_super(s, pair_ps[s])
        if fin is not None:
            for s in pair:
                for b in cfg.blocks_of(s):
                    final_linear_block(cfg, nc, fin, fin_srcs, b)
        for qt, hs in enumerate(cfg.AG_SUPER):
            if hs <= max(pair) and qt not in fired and hooks:
                fired.add(qt)
                hooks[qt]()


def make_inputs(cfg, meta, per_core, x, W, b):
    import ml_dtypes
    x = np.asarray(x, dtype=np.float32)
    W = np.asarray(W, dtype=np.float32)
    b = np.asarray(b, dtype=np.float32)
    # w_lhsT[f, k, o] = W[o, f*K + k]
    wl = W.reshape(cfg.F, cfg.F, cfg.K).transpose(1, 2, 0)  # W[o, f, k] -> [f,k,o]
    wl = np.ascontiguousarray(wl).reshape(cfg.F, cfg.K * cfg.F)
    wl = wl.astype(ml_dtypes.bfloat16)
    xb = x.astype(ml_dtypes.bfloat16)
    x_tabs = cfg.perm_quarters(xb)
    in_maps = []
    for cid in range(cfg.CORES):
        im = {
            "x_shard": np.ascontiguousarray(
                x[cid * cfg.RPC:(cid + 1) * cfg.RPC]),
            "x_bsh": np.ascontiguousarray(
                xb[cid * cfg.RPC:(cid + 1) * cfg.RPC]),
            "idx_all": per_core[cid]["idx_all"],
            "dv_all": per_core[cid]["dv_all"],
            "w_lhsT": wl,
            "b_row": b.reshape(1, cfg.F),
        }
        for q in range(4):
            im[f"x_tab{q}"] = x_tabs[q]
        in_maps.append(im)
    return in_maps


def kernel(x, lap_rows, lap_cols, lap_vals, W, b, k):
    cfg = Cfg()
    assert int(k) == cfg.K
    meta, per_core = preprocess(cfg, lap_rows, lap_cols, lap_vals)
    nc = build(cfg, meta)
    in_maps = make_inputs(cfg, meta, per_core, x, W, b)
    res = bass_utils.run_bass_kernel_spmd(
        nc, in_maps, core_ids=list(range(cfg.CORES)))
    out = np.concatenate([res.results[c]["out_shard"]
                          for c in range(cfg.CORES)], axis=0)
    return out.astype(np.float32)



# revision 10
# speedup vs baseline: 1.0579x; 1.0579x over previous
"""ChebNetConv (K=4) Bass kernel for 8 trn2 NeuronCores.

Strategy (1D row partitioning per sharding hint):
  - Nodes sharded across 8 cores (12500 rows each). Each SpMM step computes
    the core's own output rows; full neighbor tables (x / T1 / T2) are
    available to every core (x as replicated input; T1/T2 via AllGather).
  - Neighbor tables are split into four QUARTERS (by producing dest-block
    ranges, block-aligned) with separate AllGathers fired as soon as each
    quarter's rows are evicted (after superblocks 2/5/8/12).  Chunk c of
    the next step's gathers reads quarter-table c, so only the last
    quarter can stall, and superblock PAIRS process chunks {0,1,2} of
    both supers before chunk {3} to cover that latency.
  - SpMM core: edges grouped by (dest block of 128 rows, src quarter) and
    padded to batches of 128.  Per (superblock, chunk) region: dma_gather
    pulls source rows (256B bf16) into SBUF G[128e, nb, 128f]; selector
    tiles S[128e, 128d, nb] (Laplacian values at (e, dest-in-block)) are
    built ON-CHIP by two batched DVE passes (2x perf mode: q innermost,
    materialized iota constant):
        S = is_equal(iotaC, dst_bcast) * val_bcast
    PE matmuls accumulate S[:,:,q].T @ G[:,q,:] into dest-block PSUM.
  - Chebyshev recurrence (T2 = 2*L@T1 - T0) fused into batched PSUM
    eviction (one scalar_tensor_tensor per 4-block PSUM bank); bf16
    copies feed the AllGathers; T2/T3 skip the f32 DRAM round trip.
  - Final linear runs inline with step 3 (per super-pair): cheb tiles are
    loaded via HWDGE DMA-transpose from the bf16 shards (no PE transpose),
    K matmuls in bf16 against W slices accumulate in PSUM, bias added on
    DVE via partition-broadcast.
"""

import itertools

import numpy as np

import concourse.bacc as bacc
import concourse.bass as bass
import concourse.mybir as mybir
import concourse.tile as tile
from concourse import bass_utils
from concourse.bass import ds
from concourse.masks import make_identity

P = 128


class Cfg:
    def __init__(self, n_nodes=100000, f=128, k=4, cores=8, superb=8):
        assert n_nodes % cores == 0
        self.N = n_nodes
        self.F = f
        self.K = k
        self.CORES = cores
        self.RPC = n_nodes // cores            # rows per core
        self.NBLK = -(-self.RPC // P)          # dest blocks per core
        self.SUPER = superb                    # dest blocks per super-block
        self.NSUPER = -(-self.NBLK // superb)
        # quarter split (block-aligned) for pipelined AllGathers
        self.QBLK0 = [0, 24, 48, 72]           # first block of each quarter
        self.QBLK1 = [24, 48, 72, self.NBLK]   # one-past-last block
        self.QROWS = [min(b1 * P, self.RPC) - b0 * P
                      for b0, b1 in zip(self.QBLK0, self.QBLK1)]  # local rows
        self.QSTART = [b0 * P for b0 in self.QBLK0]
        self.TQ = [cores * r for r in self.QROWS]  # quarter-table rows
        for t in self.TQ:
            assert t <= 32767                  # int16 gather-idx limit
        self.NCHUNK = 4                        # chunk c == quarter c
        # AG for quarter q fires once superblock AG_SUPER[q] has evicted
        self.AG_SUPER = [2, 5, 8, self.NSUPER - 1]
        # superblock pairs (chunk phases {0,1,2} then {3} within a pair)
        self.PAIRS = [tuple(range(i, min(i + 2, self.NSUPER)))
                      for i in range(0, self.NSUPER, 2)]

    def blocks_of(self, s):
        return range(s * self.SUPER, min(self.NBLK, (s + 1) * self.SUPER))

    def nrows_of(self, b):
        return min(P, self.RPC - b * P)

    def quarter_of(self, b):
        for q in range(4):
            if b < self.QBLK1[q]:
                return q
        raise AssertionError(b)

    def map_cols(self, cols):
        """Map original node ids -> (chunk, idx-within-chunk) in the
        quarter-table layout."""
        o = cols // self.RPC
        loc = cols % self.RPC
        chk = np.zeros(len(cols), dtype=np.int64)
        src = np.zeros(len(cols), dtype=np.int64)
        for q in range(4):
            m = (loc >= self.QSTART[q]) & (loc < self.QSTART[q] + self.QROWS[q])
            chk[m] = q
            src[m] = o[m] * self.QROWS[q] + (loc[m] - self.QSTART[q])
        return chk, src

    def perm_quarters(self, full):
        """Split a [N, F] array into the 4 permuted quarter-table layouts."""
        outs = []
        for q in range(4):
            outs.append(np.ascontiguousarray(np.concatenate(
                [full[o * self.RPC + self.QSTART[q]:
                      o * self.RPC + self.QSTART[q] + self.QROWS[q]]
                 for o in range(self.CORES)], axis=0)))
        return outs


def preprocess(cfg, rows, cols, vals):
    """Build per-core gather-index and (dst, val) selector streams.

    Returns (meta, per_core) where meta has compile-time batch counts
    (identical across cores) and per_core[c] = dict of input arrays.
    """
    rows = np.asarray(rows).astype(np.int64)
    cols = np.asarray(cols).astype(np.int64)
    vals = np.asarray(vals).astype(np.float32)

    core = rows // cfg.RPC
    loc = rows % cfg.RPC
    blk = loc // P
    dst = loc % P
    chk, src = cfg.map_cols(cols)

    counts = np.zeros((cfg.CORES, cfg.NBLK, cfg.NCHUNK), dtype=np.int64)
    np.add.at(counts, (core, blk, chk), 1)
    NB = np.maximum(1, -(-counts.max(axis=0) // P))  # [NBLK, NCHUNK] batches
    # force even per-(super, chunk) batch counts (4B-aligned DVE runs)
    for s in range(cfg.NSUPER):
        bl = list(cfg.blocks_of(s))
        for c in range(cfg.NCHUNK):
            if sum(NB[b, c] for b in bl) % 2:
                NB[bl[-1], c] += 1

    # slot (b, c) capacity NB[b,c]*128; slot start offsets in padded edge space,
    # ordered (super, chunk, block-in-super, batch)
    slot_start = np.zeros((cfg.NBLK, cfg.NCHUNK), dtype=np.int64)
    call_start = {}          # (s, c) -> padded-edge offset of the gather call
    call_nbatch = {}         # (s, c) -> total batches in call
    off = 0
    for s in range(cfg.NSUPER):
        for c in range(cfg.NCHUNK):
            call_start[(s, c)] = off
            nb = 0
            for b in cfg.blocks_of(s):
                slot_start[b, c] = off
                off += NB[b, c] * P
                nb += NB[b, c]
            call_nbatch[(s, c)] = nb
    tot_pad = off

    meta = dict(NB=NB, call_start=call_start, call_nbatch=call_nbatch,
                tot_pad=tot_pad)

    import ml_dtypes
    per_core = []
    for cid in range(cfg.CORES):
        m = core == cid
        key = (blk[m] * cfg.NCHUNK + chk[m])
        order = np.argsort(key, kind="stable")
        kb, kc, ksrc, kdst, kval = (blk[m][order], chk[m][order],
                                    src[m][order], dst[m][order],
                                    vals[m][order])
        # rank within slot
        cnt = counts[cid].reshape(-1)
        slot_flat = kb * cfg.NCHUNK + kc
        starts = np.zeros(cfg.NBLK * cfg.NCHUNK, dtype=np.int64)
        starts[1:] = np.cumsum(cnt)[:-1]
        rank = np.arange(len(kb)) - starts[slot_flat]
        pos = slot_start.reshape(-1)[slot_flat] + rank  # padded global position

        idx_flat = np.zeros(tot_pad, dtype=np.int16)
        idx_flat[pos] = ksrc.astype(np.int16)
        # compact selector stream: per padded edge (dst, val) bf16 pairs
        dv = np.zeros((tot_pad, 2), dtype=ml_dtypes.bfloat16)
        dv[pos, 0] = kdst.astype(ml_dtypes.bfloat16)
        dv[pos, 1] = kval.astype(ml_dtypes.bfloat16)

        # idx DMA layout: per call, [128, 8*nb] with idx j at
        # [16g + j%16, j//16] for replica groups g=0..7
        idx_parts = []
        dv_parts = []
        for s in range(cfg.NSUPER):
            for c in range(cfg.NCHUNK):
                o = call_start[(s, c)]
                nb = call_nbatch[(s, c)]
                iv = idx_flat[o:o + nb * P]            # [nb*128]
                arr = iv.reshape(-1, 16).T             # [16, 8*nb]
                idx_parts.append(np.tile(arr, (8, 1)).reshape(-1))
                # dv layout per call: [128p, 2, nb] (q innermost)
                dvv = dv[o:o + nb * P].reshape(nb, P, 2)
                dv_parts.append(np.ascontiguousarray(
                    dvv.transpose(1, 2, 0)).reshape(-1))
        per_core.append(dict(
            idx_all=np.concatenate(idx_parts),
            dv_all=np.concatenate(dv_parts),
        ))
    return meta, per_core


def emulate(cfg, meta, per_core, full_tab):
    """Numpy emulation of the on-device SpMM. full_tab: [N, F] table in
    ORIGINAL node order. Returns per-core [RPC, F] segment sums."""
    NB = meta["NB"]
    chunk_tabs = cfg.perm_quarters(full_tab)
    outs = []
    for cid in range(cfg.CORES):
        pc = per_core[cid]
        out = np.zeros((cfg.RPC, cfg.F), dtype=np.float32)
        iofs = 0
        sofs = 0
        for s in range(cfg.NSUPER):
            for c in range(cfg.NCHUNK):
                nb = meta["call_nbatch"][(s, c)]
                w8 = nb * 8
                idx_tile = pc["idx_all"][iofs:iofs + 128 * w8].reshape(128, w8)
                iofs += 128 * w8
                n = nb * P
                unwrapped = idx_tile[:16, :].T.reshape(-1)[:n].astype(np.int64)
                g = chunk_tabs[c][unwrapped]             # [n, F]
                g = g.reshape(nb, P, cfg.F)
                dvt = pc["dv_all"][sofs:sofs + 128 * 2 * nb].reshape(128, 2, nb)
                sofs += 128 * 2 * nb
                q0 = 0
                for b in cfg.blocks_of(s):
                    for q in range(NB[b, c]):
                        dd = dvt[:, 0, q0 + q].astype(np.int64)
                        vv = dvt[:, 1, q0 + q].astype(np.float32)
                        S = np.zeros((P, P), dtype=np.float32)
                        S[np.arange(P), dd] = vv
                        G = g[q0 + q].astype(np.float32)
                        out[b * P:b * P + cfg.nrows_of(b), :] += \
                            (S.T @ G)[:cfg.nrows_of(b)]
                    q0 += NB[b, c]
        outs.append(out)
    return outs


def build(cfg, meta):
    """Build the Bass program. Returns nc."""
    f32 = mybir.dt.float32
    bf16 = mybir.dt.bfloat16
    nc = bacc.Bacc("TRN2", target_bir_lowering=False, debug=False,
                   num_devices=cfg.CORES, num_swdge_queues=4)

    x_tabq = [nc.dram_tensor(f"x_tab{q}", [cfg.TQ[q], cfg.F], bf16,
                             kind="ExternalInput") for q in range(4)]
    x_shard = nc.dram_tensor("x_shard", [cfg.RPC, cfg.F], f32,
                             kind="ExternalInput")
    x_bsh = nc.dram_tensor("x_bsh", [cfg.RPC, cfg.F], bf16,
                           kind="ExternalInput")
    idx_in = nc.dram_tensor("idx_all", [meta["tot_pad"] * 8], mybir.dt.int16,
                            kind="ExternalInput")
    dv_in = nc.dram_tensor("dv_all", [meta["tot_pad"] * 2], bf16,
                           kind="ExternalInput")
    w_in = nc.dram_tensor("w_lhsT", [cfg.F, cfg.K * cfg.F], bf16,
                          kind="ExternalInput")
    b_in = nc.dram_tensor("b_row", [1, cfg.F], f32, kind="ExternalInput")
    out_shard = nc.dram_tensor("out_shard", [cfg.RPC, cfg.F], f32,
                               kind="ExternalOutput")

    rg = [list(range(cfg.CORES))]
    qrr = itertools.count()  # global gather-queue round robin
    NBMAX = max(meta["call_nbatch"].values())

    with tile.TileContext(nc) as tc:
        with tc.tile_pool(name="dram", bufs=1, space="DRAM") as dram:
            t1_shard = dram.tile([cfg.RPC, cfg.F], f32, tag="t1s")
            t1_bq = [dram.tile([cfg.QROWS[q], cfg.F], bf16, tag=f"t1b{q}",
                               name=f"t1b{q}") for q in range(4)]
            t2_bq = [dram.tile([cfg.QROWS[q], cfg.F], bf16, tag=f"t2b{q}",
                               name=f"t2b{q}") for q in range(4)]
            t3_bsh = dram.tile([cfg.RPC, cfg.F], bf16, tag="t3b")
            t1_tabq = [dram.tile([cfg.TQ[q], cfg.F], bf16, tag=f"t1t{q}",
                                 name=f"t1t{q}", addr_space="Shared")
                       for q in range(4)]
            t2_tabq = [dram.tile([cfg.TQ[q], cfg.F], bf16, tag=f"t2t{q}",
                                 name=f"t2t{q}", addr_space="Shared")
                       for q in range(4)]

            def ag(bsh, tab):
                nc.gpsimd.collective_compute(
                    "AllGather", mybir.AluOpType.bypass, replica_groups=rg,
                    ins=[bsh[:].opt()], outs=[tab[:].opt()])

            with (
                tc.tile_pool(name="const", bufs=1) as constp,
                tc.tile_pool(name="gpool", bufs=4) as gpool,
                tc.tile_pool(name="spool", bufs=3) as spool,
                tc.tile_pool(name="ipool", bufs=5) as ipool,
                tc.tile_pool(name="dvpool", bufs=5) as dvpool,
                tc.tile_pool(name="psum", bufs=2, space="PSUM") as pspool,
                tc.tile_pool(name="ev", bufs=4) as evpool,
                tc.tile_pool(name="fconst", bufs=1) as fconst,
                tc.tile_pool(name="ftrans", bufs=8) as ftrans,
                tc.tile_pool(name="fpsum", bufs=2, space="PSUM") as fpsum,
                tc.tile_pool(name="fout", bufs=3) as foutp,
            ):
                iotaC = constp.tile([P, P, NBMAX], bf16)
                nc.gpsimd.iota(iotaC[:], pattern=[[1, P], [0, NBMAX]], base=0,
                               channel_multiplier=0,
                               allow_small_or_imprecise_dtypes=True)
                wt = fconst.tile([cfg.F, cfg.K, cfg.F], bf16)
                nc.sync.dma_start(wt[:], w_in[:].rearrange(
                    "f (k o) -> f k o", k=cfg.K))
                brow = fconst.tile([1, cfg.F], f32)
                nc.sync.dma_start(brow[:], b_in[:])
                ones = fconst.tile([1, P], f32)
                nc.vector.memset(ones[:], 1.0)

                fin = dict(ftrans=ftrans, fpsum=fpsum, foutp=foutp, wt=wt,
                           brow=brow, ones=ones, out_shard=out_shard)
                for step in (1, 2, 3):
                    tabs = {1: [t[:] for t in x_tabq],
                            2: [t[:] for t in t1_tabq],
                            3: [t[:] for t in t2_tabq]}[step]
                    prev = {1: None, 2: x_shard, 3: t1_shard}[step]
                    dst = {1: t1_shard, 2: None, 3: None}[step]
                    bq = {1: t1_bq, 2: t2_bq, 3: None}[step]
                    hooks = {}
                    if step == 1:
                        hooks = {q: (lambda q=q: ag(t1_bq[q], t1_tabq[q]))
                                 for q in range(4)}
                    elif step == 2:
                        hooks = {q: (lambda q=q: ag(t2_bq[q], t2_tabq[q]))
                                 for q in range(4)}
                    fin_step = fin if step == 3 else None
                    fin_srcs = ([x_bsh[:], t1_bq, t2_bq, t3_bsh[:]]
                                if step == 3 else None)
                    spmm_step(cfg, meta, nc, tc, gpool, spool, ipool, dvpool,
                              pspool, evpool, idx_in, dv_in, iotaC, tabs,
                              prev, dst, qrr, bq, t3_bsh if step == 3 else None,
                              hooks, fin_step, fin_srcs)

    nc.compile()
    return nc


def final_linear_block(cfg, nc, fin, fin_srcs, b):
    """Inline final linear for one dest block (step-3 tail)."""
    f32 = mybir.dt.float32
    bf16 = mybir.dt.bfloat16
    nrows = cfg.nrows_of(b)
    r0 = b * P
    opsum = fin["fpsum"].tile([P, cfg.F], f32, tag="opsum")
    for k in range(cfg.K):
        src = fin_srcs[k]
        if isinstance(src, list):  # quarter tiles
            q = cfg.quarter_of(b)
            reg = src[q][(b - cfg.QBLK0[q]) * P:
                         (b - cfg.QBLK0[q]) * P + nrows, :]
        else:
            reg = src[r0:r0 + nrows, :]
        cT = fin["ftrans"].tile([cfg.F, P], bf16, tag="cT")
        # scalar (ACT) HWDGE ring: keeps the transpose stream off the sync
        # ring that carries the idx/dv/pv loads (head-of-line blocking).
        if nrows == P:
            nc.scalar.dma_start_transpose(cT[:, :nrows], reg)
        else:
            nc.scalar.dma_start(cT[:, :nrows], reg.rearrange("a b -> b a"))
        nc.tensor.matmul(opsum[:nrows, :], cT[:, :nrows],
                         fin["wt"][:, k, :], start=(k == 0), stop=False)
    nc.tensor.matmul(opsum[:nrows, :], fin["ones"][:1, :nrows],
                     fin["brow"][:1, :], start=False, stop=True)
    ot = fin["foutp"].tile([P, cfg.F], f32, tag="ot")
    nc.vector.tensor_copy(ot[:nrows, :], opsum[:nrows, :])
    nc.scalar.dma_start(fin["out_shard"][r0:r0 + nrows, :], ot[:nrows, :])


def spmm_step(cfg, meta, nc, tc, gpool, spool, ipool, dvpool, pspool, evpool,
              idx_in, dv_in, iotaC, tabs, prev, dst, qrr, bq, t3b, hooks,
              fin, fin_srcs):
    NB = meta["NB"]
    f32 = mybir.dt.float32
    bf16 = mybir.dt.bfloat16
    eq = mybir.AluOpType.is_equal
    mul = mybir.AluOpType.mult
    sub = mybir.AluOpType.subtract
    base_i = {}
    base_s = {}
    off_i = 0
    off_s = 0
    for s in range(cfg.NSUPER):
        for c in range(cfg.NCHUNK):
            nb = meta["call_nbatch"][(s, c)]
            base_i[(s, c)] = off_i
            base_s[(s, c)] = off_s
            off_i += P * nb * 8
            off_s += P * nb * 2

    def do_chunk(s, c, ps):
        blocks = list(cfg.blocks_of(s))
        src = tabs[c]
        nb = meta["call_nbatch"][(s, c)]
        w8 = nb * 8
        iofs = base_i[(s, c)]
        sofs = base_s[(s, c)]
        ix = ipool.tile([P, w8], mybir.dt.int16, tag="ix")
        nc.sync.dma_start(
            ix[:], idx_in[iofs:iofs + P * w8].rearrange("(p w) -> p w", p=P))
        dv = dvpool.tile([P, 2, nb], bf16, tag="dv")
        nc.sync.dma_start(
            dv[:], dv_in[sofs:sofs + P * nb * 2].rearrange(
                "(p t b) -> p t b", p=P, t=2))
        g = gpool.tile([P, nb, cfg.F], bf16, tag="G")
        MAXB = 8
        for b0 in range(0, nb, MAXB):
            b1 = min(nb, b0 + MAXB)
            nc.gpsimd.dma_gather(
                g[:, b0:b1, :], src, ix[:, b0 * 8:b1 * 8],
                (b1 - b0) * P, (b1 - b0) * P, cfg.F,
                single_packet=(b1 - b0) <= 8, queue_num=next(qrr) % 4)
        # batched on-chip selector build (2 DVE passes, q innermost -> 2x)
        sl = spool.tile([P, P, nb], bf16, tag="S")
        nc.vector.tensor_tensor(
            sl[:], iotaC[:, :, :nb], dv[:, 0:1, :].broadcast_to([P, P, nb]),
            op=eq)
        nc.vector.tensor_tensor(
            sl[:], sl[:], dv[:, 1:2, :].broadcast_to([P, P, nb]), op=mul)
        q0 = 0
        for bi, b in enumerate(blocks):
            pt = ps[bi // 4][:, bi % 4, :]
            # one accumulation group per PSUM bank: start clears
            # has_written bank-wide, so only the first matmul into the
            # bank may set it; per-element has_written handles the
            # disjoint block slices.
            last_in_bank = bi % 4 == 3 or bi == len(blocks) - 1
            for q in range(NB[b, c]):
                nc.tensor.matmul(
                    pt, sl[:, :, q0 + q], g[:, q0 + q, :],
                    start=(c == 0 and q == 0 and bi % 4 == 0),
                    stop=(c == cfg.NCHUNK - 1 and q == NB[b, c] - 1
                          and last_in_bank),
                    skip_group_check=True)
            q0 += NB[b, c]

    def evict_super(s, ps):
        blocks = list(cfg.blocks_of(s))
        gi = 0
        while gi < len(blocks):
            grp = blocks[gi:gi + 4]
            pst = ps[gi // 4]
            full = len(grp) == 4 and all(cfg.nrows_of(b) == P for b in grp)
            b0 = grp[0]
            r0 = b0 * P
            if full:
                ev4 = evpool.tile([P, 4, cfg.F], f32, tag="ev4")
                if prev is None:
                    nc.vector.tensor_copy(ev4[:], pst[:])
                else:
                    pv4 = evpool.tile([P, 4, cfg.F], f32, tag="pv4")
                    nc.sync.dma_start(pv4[:], prev[r0:r0 + 4 * P, :].rearrange(
                        "(g p) f -> p g f", p=P))
                    nc.vector.scalar_tensor_tensor(
                        ev4[:], pst[:], 2.0, pv4[:], op0=mul, op1=sub)
                if dst is not None:
                    nc.scalar.dma_start(
                        dst[r0:r0 + 4 * P, :].rearrange(
                            "(g p) f -> p g f", p=P), ev4[:])
                evb = evpool.tile([P, 4, cfg.F], bf16, tag="evb4")
                nc.vector.tensor_copy(evb[:], ev4[:])
                if bq is not None:
                    qt = cfg.quarter_of(b0)
                    tr0 = r0 - cfg.QSTART[qt]
                    tgt = bq[qt]
                else:
                    qt = None
                    tr0 = r0
                    tgt = t3b
                nc.scalar.dma_start(
                    tgt[tr0:tr0 + 4 * P, :].rearrange(
                        "(g p) f -> p g f", p=P), evb[:])
                gi += 4
            else:
                for j, b in enumerate(grp):
                    pt = pst[:, j, :]
                    nrows = cfg.nrows_of(b)
                    r0 = b * P
                    ev = evpool.tile([P, cfg.F], f32, tag="ev")
                    if prev is None:
                        nc.vector.tensor_copy(ev[:nrows, :], pt[:nrows, :])
                    else:
                        pv = evpool.tile([P, cfg.F], f32, tag="pv")
                        nc.sync.dma_start(pv[:nrows, :],
                                          prev[r0:r0 + nrows, :])
                        nc.vector.scalar_tensor_tensor(
                            ev[:nrows, :], pt[:nrows, :], 2.0, pv[:nrows, :],
                            op0=mul, op1=sub)
                    if dst is not None:
                        nc.scalar.dma_start(dst[r0:r0 + nrows, :],
                                            ev[:nrows, :])
                    evb = evpool.tile([P, cfg.F], bf16, tag="evb")
                    nc.vector.tensor_copy(evb[:nrows, :], ev[:nrows, :])
                    if bq is not None:
                        qt = cfg.quarter_of(b)
                        tgt, tr0 = bq[qt], r0 - cfg.QSTART[qt]
                    else:
                        tgt, tr0 = t3b, r0
                    nc.scalar.dma_start(tgt[tr0:tr0 + nrows, :],
                                        evb[:nrows, :])
                gi += len(grp)

    fired = set()
    for pair in cfg.PAIRS:
        pair_ps = {}
        for s in pair:
            blocks = list(cfg.blocks_of(s))
            pair_ps[s] = [pspool.tile([P, 4, cfg.F], f32, tag=f"ps{i}",
                                      name=f"ps{i}_{s}")
                          for i in range(-(-len(blocks) // 4))]
        for c in (0, 1, 2):
            for s in pair:
                do_chunk(s, c, pair_ps[s])
        for s in pair:
            do_chunk(s, 3, pair_ps[s])
        for s in pair:
            evict_super(s, pair_ps[s])
        if fin is not None:
            for s in pair:
                for b in cfg.blocks_of(s):
                    final_linear_block(cfg, nc, fin, fin_srcs, b)
        for qt, hs in enumerate(cfg.AG_SUPER):
            if hs <= max(pair) and qt not in fired and hooks:
                fired.add(qt)
                hooks[qt]()


def make_inputs(cfg, meta, per_core, x, W, b):
    import ml_dtypes
    x = np.asarray(x, dtype=np.float32)
    W = np.asarray(W, dtype=np.float32)
    b = np.asarray(b, dtype=np.float32)
    # w_lhsT[f, k, o] = W[o, f*K + k]
    wl = W.reshape(cfg.F, cfg.F, cfg.K).transpose(1, 2, 0)  # W[o, f, k] -> [f,k,o]
    wl = np.ascontiguousarray(wl).reshape(cfg.F, cfg.K * cfg.F)
    wl = wl.astype(ml_dtypes.bfloat16)
    xb = x.astype(ml_dtypes.bfloat16)
    x_tabs = cfg.perm_quarters(xb)
    in_maps = []
    for cid in range(cfg.CORES):
        im = {
            "x_shard": np.ascontiguousarray(
                x[cid * cfg.RPC:(cid + 1) * cfg.RPC]),
            "x_bsh": np.ascontiguousarray(
                xb[cid * cfg.RPC:(cid + 1) * cfg.RPC]),
            "idx_all": per_core[cid]["idx_all"],
            "dv_all": per_core[cid]["dv_all"],
            "w_lhsT": wl,
            "b_row": b.reshape(1, cfg.F),
        }
        for q in range(4):
            im[f"x_tab{q}"] = x_tabs[q]
        in_maps.append(im)
    return in_maps


def kernel(x, lap_rows, lap_cols, lap_vals, W, b, k):
    cfg = Cfg()
    assert int(k) == cfg.K
    meta, per_core = preprocess(cfg, lap_rows, lap_cols, lap_vals)
    nc = build(cfg, meta)
    in_maps = make_inputs(cfg, meta, per_core, x, W, b)
    res = bass_utils.run_bass_kernel_spmd(
        nc, in_maps, core_ids=list(range(cfg.CORES)))
    out = np.concatenate([res.results[c]["out_shard"]
                          for c in range(cfg.CORES)], axis=0)
    return out.astype(np.float32)

